# revision 1
# baseline (speedup 1.0000x reference)
"""Causal self-attention with ALiBi, sharded over 8 TRN2 NeuronCores.

Sharding: core c -> batch b = c//4, head group g = c%4 (4 heads each).
Each core computes QKV projection for its heads, causal attention, and the
partial output projection (w_proj rows of its heads). Host sums the 4
partials per batch and adds b_proj.

Kernel math tricks (all folded into matmuls so softmax is one exp pass):
  - scores are computed TRANSPOSED (s on partitions, t free) so exp(S^T)=P^T
    lands exactly in the lhsT layout the P@V matmul needs.
  - ALiBi bias slope*s, the stability offset -(slope*t + c), and the /sqrt(D)
    scale are folded into 3 extra contraction rows of the QK^T matmul
    (q' = [q/8, 1, 1, -(slope*t+c)], k' = [k, hi, lo, 1] with hi+lo an exact
    split of slope*s to survive f32r rounding).
  - V is augmented with a ones column so the softmax denominator appears as
    row 64 of the (unnormalized) y^T accumulator.
  - normalization commutes with the head-dim contraction, applied via
    reciprocal + partition broadcast before the output projection.
All matmuls run in f32r (single-pass fp32, ~1e-4 rel err).
"""

import numpy as np

B, T, C, H = 2, 2048, 1024, 16
D = C // H          # 64
HL = 4              # heads per core
NCORES = 8
COFF = 8.0          # softmax stability offset

_prog_cache = {}


def _round_keep9(x):
    """RNE to 9 explicit mantissa bits (exactly representable in f32r)."""
    b = np.asarray(x, np.float32).view(np.uint32)
    half = np.uint32(1 << 13)
    mask = np.uint32(0xFFFFFFFF) << 14
    return ((b + half) & mask).view(np.float32)


def _build_program():
    import concourse.bass as bass  # noqa: F401
    import concourse.mybir as mybir
    import concourse.tile as tile
    from concourse import bacc

    f32 = mybir.dt.float32
    f32r = mybir.dt.float32r
    EXP = mybir.ActivationFunctionType.Exp
    CPY = mybir.ActivationFunctionType.Copy

    nc = bacc.Bacc("TRN2", target_bir_lowering=False, num_devices=NCORES)

    x_in = nc.declare_dram_parameter("x", [T, C], f32r, isOutput=False)
    wqk_in = nc.declare_dram_parameter("wqk", [C, 512], f32r, isOutput=False)
    wv_in = nc.declare_dram_parameter("wv", [C, 256], f32r, isOutput=False)
    wp_in = nc.declare_dram_parameter("wp", [256, C], f32r, isOutput=False)
    bqk_in = nc.declare_dram_parameter("bqk", [128, 4], f32, isOutput=False)
    bv_in = nc.declare_dram_parameter("bv", [1, 256], f32r, isOutput=False)
    # aug rows per head: [.., 29:32, :] = the 3 aug rows ([1,1,qaug] q-side,
    # [khi,klo,1] k-side); rows 0:29 are zeros (odd-head padding).
    augq_in = nc.declare_dram_parameter("augq", [HL, 32, T], f32r, isOutput=False)
    augk_in = nc.declare_dram_parameter("augk", [HL, 32, T], f32r, isOutput=False)
    out_dram = nc.declare_dram_parameter("out", [T, C], f32, isOutput=True)

    with tile.TileContext(nc) as tc:
        with (
            tc.tile_pool(name="persist", bufs=1) as pp,
            tc.tile_pool(name="consts", bufs=1) as cp,
        ):
            # ---- constants / weights ----
            from concourse.masks import make_identity

            ident = cp.tile([128, 128], f32)
            make_identity(nc, ident)
            identr = cp.tile([128, 128], f32r)
            nc.vector.tensor_copy(identr, ident)

            # prefetch the first t-super of x before anything else so the
            # transposes (first PE work) start as early as possible
            p2 = tc.alloc_tile_pool(name="ph2", bufs=2)
            p2pt = tc.alloc_tile_pool(name="ph2pt", bufs=2)
            p3 = tc.alloc_tile_pool(name="ph3", bufs=2)
            ps2a = tc.alloc_tile_pool(name="ps2a", bufs=2, space="PSUM")
            ps2b = tc.alloc_tile_pool(name="ps2b", bufs=1, space="PSUM")
            p1a = tc.alloc_tile_pool(name="ph1a", bufs=1)
            p1b = tc.alloc_tile_pool(name="ph1b", bufs=1)
            psP = tc.alloc_tile_pool(name="psP", bufs=2, space="PSUM")
            xn0 = []
            for k in range(4):
                xt_ = p1a.tile([128, C], f32r, tag=f"xnat{k}")
                nc.sync.dma_start(out=xt_, in_=x_in[128 * k:128 * (k + 1), :])
                xn0.append(xt_)

            wqk_sb = [cp.tile([128, 512], f32r, name=f"wqk{c}", tag=f"wqk{c}") for c in range(8)]
            for c in range(8):
                nc.sync.dma_start(out=wqk_sb[c], in_=wqk_in[128 * c:128 * (c + 1), :])
            wv_sb = [cp.tile([128, 256], f32r, name=f"wv{c}", tag=f"wv{c}") for c in range(8)]
            for c in range(8):
                nc.sync.dma_start(out=wv_sb[c], in_=wv_in[128 * c:128 * (c + 1), :])
            bqk_sb = cp.tile([128, 4], f32)
            nc.sync.dma_start(out=bqk_sb, in_=bqk_in[:, :])
            bv_sb = cp.tile([1, 256], f32r)
            nc.sync.dma_start(out=bv_sb, in_=bv_in[:, :])
            ones_t = cp.tile([1, 128], f32r)
            nc.vector.memset(ones_t.bitcast(f32), 1.0)

            # ---- persistent attention operands ----
            # Q'/K' per head: [128, T]. Even local head: rows 0-63 head data,
            # rows 64-66 augs. Odd local head: rows 61-63 augs, 64-127 data.
            QP = [pp.tile([128, T], f32r, name=f"QP{h}", tag=f"QP{h}") for h in range(HL)]
            KP = [pp.tile([128, T], f32r, name=f"KP{h}", tag=f"KP{h}") for h in range(HL)]
            # V' per s-block: [128, HL, 65] (cols 0-63 = v, col 64 = ones)
            VP = [pp.tile([128, HL, 65], f32r, name=f"VP{j}", tag=f"VP{j}") for j in range(16)]
            # normalized y^T stacked per head pair: [128, T]
            PAIR = [pp.tile([128, T], f32r, name=f"PAIR{p}", tag=f"PAIR{p}") for p in range(2)]

            for h in range(HL):
                if h % 2 == 0:
                    # rows 64-66 = augs; contraction slice [0:67]
                    nc.sync.dma_start(out=QP[h][64:67, :], in_=augq_in[h, 29:32, :])
                    nc.sync.dma_start(out=KP[h][64:67, :], in_=augk_in[h, 29:32, :])
                else:
                    # contraction slice [0:128]: rows 0-60 zero, 61-63 augs,
                    # 64-127 data (zero rows cost nothing: PE time ~ N only)
                    nc.vector.memset(QP[h][0:32, :].bitcast(f32), 0.0)
                    nc.vector.memset(KP[h][0:32, :].bitcast(f32), 0.0)
                    nc.sync.dma_start(out=QP[h][32:64, :], in_=augq_in[h, :, :])
                    nc.sync.dma_start(out=KP[h][32:64, :], in_=augk_in[h, :, :])
            for j in range(16):
                nc.vector.memset(VP[j][:, :, 64:65].bitcast(f32), 1.0)

            # ===== interleaved pipeline: projections feed attention =====
            # PSUM budget (8 banks): p1 shared proj staging (2) + scores (4)
            # + y accumulators (2); after phase-1 release, fp takes p1's banks.
            psF = [None]

            if True:
                wp_sb = [p3.tile([128, C], f32r, name=f"wp{p}", tag=f"wp{p}") for p in range(2)]
                for p in range(2):
                    nc.sync.dma_start(out=wp_sb[p], in_=wp_in[128 * p:128 * (p + 1), :])

                def emit_ts(ts):
                    if ts == 0:
                        xn = xn0
                    else:
                        xn = []
                        for k in range(4):
                            t0 = 512 * ts + 128 * k
                            xt_ = p1a.tile([128, C], f32r, tag=f"xnat{k}")
                            nc.sync.dma_start(out=xt_, in_=x_in[t0:t0 + 128, :])
                            xn.append(xt_)
                    xtc = []
                    for c in range(8):
                        tp = psP.tile([128, 512], f32, tag="p1")
                        for k in range(4):
                            nc.tensor.transpose(
                                tp[:, 128 * k:128 * (k + 1)].bitcast(f32r),
                                xn[k][:, 128 * c:128 * (c + 1)],
                                identr,
                            )
                        xc = p1b.tile([128, 512], f32r, tag=f"xtc{c}")
                        nc.scalar.activation(xc, tp, CPY)
                        xtc.append(xc)
                    for m in range(4):
                        qk = psP.tile([128, 512], f32, tag="p1")
                        for c in range(8):
                            nc.tensor.matmul(
                                qk,
                                wqk_sb[c][:, 128 * m:128 * (m + 1)],
                                xtc[c],
                                start=(c == 0),
                                stop=(c == 7),
                            )
                        dest = QP if m < 2 else KP
                        h0 = 2 * (m % 2)
                        tsl = slice(512 * ts, 512 * (ts + 1))
                        nc.vector.tensor_scalar_add(
                            dest[h0][0:64, tsl], qk[0:64, :], bqk_sb[0:64, m:m + 1]
                        )
                        nc.vector.tensor_scalar_add(
                            dest[h0 + 1][64:128, tsl], qk[64:128, :], bqk_sb[64:128, m:m + 1]
                        )
                    for k in range(4):
                        jj = 4 * ts + k
                        vp = psP.tile([128, 512], f32, tag="p1")
                        for c in range(8):
                            nc.tensor.matmul(
                                vp[:, 0:256],
                                xtc[c][:, 128 * k:128 * (k + 1)],
                                wv_sb[c],
                                start=(c == 0),
                                stop=False,
                            )
                        nc.tensor.matmul(vp[:, 0:256], ones_t, bv_sb, start=False, stop=True)
                        nc.vector.tensor_copy(
                            VP[jj][:, :, 0:64],
                            vp[:, 0:256].rearrange("p (h d) -> p h d", h=HL),
                        )

                def normalize(h, i, yt):
                    """Evacuate Y psum, divide by denominator row, store to PAIR."""
                    ysb = p2.tile([65, 512], f32, tag="ysb")
                    nc.vector.tensor_copy(ysb, yt)  # frees the psum bank fast
                    den = p2.tile([1, 512], f32, tag="den")
                    nc.sync.dma_start(out=den, in_=ysb[64:65, :])
                    rr = p2.tile([1, 512], f32, tag="rr")
                    nc.vector.reciprocal_approx_fast(out=rr, in_=den)
                    rbc = p2.tile([64, 512], f32, tag="rbc")
                    nc.gpsimd.partition_broadcast(out_ap=rbc, in_ap=rr)
                    tsl = slice(512 * i, 512 * (i + 1))
                    if h % 2 == 0:
                        nc.vector.tensor_mul(PAIR[h // 2][0:64, tsl], ysb[0:64, :], rbc)
                    else:
                        stg = p2.tile([64, 512], f32r, tag="stg")
                        nc.vector.tensor_mul(stg, ysb[0:64, :], rbc)
                        nc.sync.dma_start(out=PAIR[h // 2][64:128, tsl], in_=stg)

                def project(i):
                    """Output projection for t-blocks of t-tile i (all heads done)."""
                    for tb in range(4 * i, 4 * i + 4):
                        fp = psF[0].tile([128, 1024], f32, tag="fp")
                        tsl = slice(128 * tb, 128 * (tb + 1))
                        for n in range(2):
                            nsl = slice(512 * n, 512 * (n + 1))
                            for p in range(2):
                                nc.tensor.matmul(
                                    fp[:, nsl],
                                    PAIR[p][:, tsl],
                                    wp_sb[p][:, nsl],
                                    start=(p == 0),
                                    stop=(p == 1),
                                )
                        ob = p3.tile([128, 1024], f32, tag="ob")
                        nc.vector.tensor_copy(ob, fp)
                        nc.sync.dma_start(out=out_dram[tsl, :], in_=ob)

                # Slot h holds global heads {h*4+g : g}; the flattest slope in
                # slot h is 2^(-2(h+1)), so keys further than DELTA[h] behind
                # the query contribute < e^-32 of the softmax mass -> skip.
                DELTA = [30 * 4 ** (h + 1) for h in range(HL)]

                def emit_att(th, hs, proj_after=()):
                    tbase = 1024 * th
                    ilo_half, ihi_half = 2 * th, 2 * th + 2
                    for h in hs:
                        rows = slice(0, 67) if h % 2 == 0 else slice(0, 128)
                        Y = {}
                        started = set()
                        for j in range(8 * th + 8):
                            i0, m = j // 4, j % 4
                            off = 128 * m
                            ilo = max(i0, ilo_half)
                            kept = [
                                i for i in range(ilo, ihi_half)
                                if 128 * j + 127 >= 512 * i - DELTA[h]
                            ]
                            if not kept:
                                continue
                            imax = kept[-1]
                            S = ps2a.tile([128, 1024], f32, tag="sc")
                            for i in kept:
                                a = 512 * i - tbase + (off if i == i0 else 0)
                                b = 512 * i - tbase + 512
                                nc.tensor.matmul(
                                    S[:, a:b],
                                    KP[h][rows, 128 * j:128 * (j + 1)],
                                    QP[h][rows, tbase + a:tbase + b],
                                    start=True,
                                    stop=True,
                                )
                            amin = 512 * kept[0] - tbase + (off if kept[0] == i0 else 0)
                            amax = 512 * imax - tbase + 512
                            PT = p2pt.tile([128, 1024], f32r, tag="pt")
                            nc.scalar.activation(PT[:, amin:amax], S[:, amin:amax], EXP)
                            if i0 >= ilo_half:
                                d0 = 512 * i0 - tbase + off
                                nc.gpsimd.affine_select(
                                    out=PT[:, d0:d0 + 128],
                                    in_=PT[:, d0:d0 + 128],
                                    compare_op=mybir.AluOpType.is_ge,
                                    fill=0.0,
                                    base=0,
                                    pattern=[[1, 128]],
                                    channel_multiplier=-1,
                                )
                            for i in sorted(kept, reverse=True):
                                if i not in Y:
                                    yt = ps2b.tile(
                                        [65, 512], f32,
                                        tag=f"yb{i % 2}", name=f"Y{h}_{i}",
                                    )
                                    Y[i] = yt
                                a = 512 * i - tbase + (off if i == i0 else 0)
                                b = 512 * i - tbase + 512
                                ya = a - (512 * i - tbase)
                                nc.tensor.matmul(
                                    Y[i][:, ya:512],
                                    VP[j][:, h, :],
                                    PT[:, a:b],
                                    start=(i not in started),
                                    stop=(j == 4 * i + 3),
                                )
                                started.add(i)
                            if j >= 3 and (j - 3) % 4 == 0:
                                i_done = (j - 3) // 4
                                if ilo_half <= i_done < ihi_half:
                                    normalize(h, i_done, Y[i_done])
                                    if h == hs[-1] and i_done in proj_after:
                                        project(i_done)

                # --- interleaved emission ---
                emit_ts(0)
                emit_ts(1)
                emit_att(0, [0, 1])
                emit_ts(2)
                emit_att(0, [2, 3])
                emit_ts(3)
                psP.release()
                p1b.release()
                p1a.release()
                psF[0] = tc.alloc_tile_pool(name="psF", bufs=1, space="PSUM")
                project(0)
                project(1)
                emit_att(1, [0, 1, 2, 3], proj_after=(2, 3))
                psF[0].release()
                ps2b.release()
                ps2a.release()
                p3.release()
                p2pt.release()
                p2.release()




    nc.finalize()
    return nc


def _get_program():
    if "nc" not in _prog_cache:
        _prog_cache["nc"] = _build_program()
    return _prog_cache["nc"]


def _prep_core_inputs(core, x, w_attn, b_attn, w_proj):
    b, g = core // 4, core % 4
    # slot i holds global head g + 4*i (slopes grouped by magnitude per slot)
    heads = [g + 4 * i for i in range(HL)]
    qc = [slice((0 * H + h) * D, (0 * H + h) * D + D) for h in heads]
    kc = [slice((1 * H + h) * D, (1 * H + h) * D + D) for h in heads]
    vc = [slice((2 * H + h) * D, (2 * H + h) * D + D) for h in heads]

    wq = np.concatenate([w_attn[:, s] for s in qc], 1) * 0.125
    wk = np.concatenate([w_attn[:, s] for s in kc], 1)
    wqk = np.concatenate([wq, wk], 1).astype(np.float32)          # [C, 512]
    wv = np.concatenate([w_attn[:, s] for s in vc], 1).astype(np.float32)
    bq = np.concatenate([b_attn[s] for s in qc]) * 0.125
    bk = np.concatenate([b_attn[s] for s in kc])
    bqk = np.concatenate([bq, bk]).astype(np.float32).reshape(4, 128).T.copy()
    bv = np.concatenate([b_attn[s] for s in vc]).astype(np.float32)[None, :]
    wp = np.concatenate([w_proj[s, :] for s in qc], 0).astype(np.float32)  # [256, C]

    slopes = 2.0 ** (-(8.0 / H) * (np.array(heads, np.float64) + 1.0))
    pos = np.arange(T, dtype=np.float64)
    kaug = slopes[:, None] * pos[None, :]                          # [HL, T]
    khi = _round_keep9(kaug)
    klo = (kaug - khi.astype(np.float64)).astype(np.float32)
    qaug = (-(kaug + COFF)).astype(np.float32)

    augq = np.zeros((HL, 32, T), np.float32)
    augq[:, 29, :] = 1.0
    augq[:, 30, :] = 1.0
    augq[:, 31, :] = qaug
    augk = np.zeros((HL, 32, T), np.float32)
    augk[:, 29, :] = khi
    augk[:, 30, :] = klo
    augk[:, 31, :] = 1.0

    return {
        "x": np.ascontiguousarray(x[b], np.float32),
        "wqk": wqk,
        "wv": wv,
        "wp": np.ascontiguousarray(wp),
        "bqk": bqk,
        "bv": bv,
        "augq": augq,
        "augk": augk,
    }


def kernel(x, w_attn, b_attn, w_proj, b_proj, _run_kwargs=None):
    from concourse.bass_utils import run_bass_kernel_spmd

    x = np.asarray(x, np.float32)
    w_attn = np.asarray(w_attn, np.float32)
    b_attn = np.asarray(b_attn, np.float32)
    w_proj = np.asarray(w_proj, np.float32)
    b_proj = np.asarray(b_proj, np.float32)

    nc = _get_program()
    in_maps = [_prep_core_inputs(c, x, w_attn, b_attn, w_proj) for c in range(NCORES)]
    res = run_bass_kernel_spmd(
        nc, in_maps, core_ids=list(range(NCORES)), **(_run_kwargs or {})
    )
    _prog_cache["last_result"] = res

    out = np.zeros((B, T, C), np.float32)
    for c in range(NCORES):
        out[c // 4] += res.results[c]["out"]
    out += b_proj[None, None, :]
    return out



# revision 14
# speedup vs baseline: 1.1925x; 1.1925x over previous
"""Causal self-attention with ALiBi, sharded over 8 TRN2 NeuronCores.

Sharding: core c -> batch b = c//4, head group g = c%4 (4 heads each).
Each core computes QKV projection for its heads, causal attention, and the
partial output projection (w_proj rows of its heads). Host sums the 4
partials per batch and adds b_proj.

v2 (vs baseline):
  - x is transposed on the host: no on-chip PE transposes or staging copies.
  - everything runs bf16 (1.0 cycle/row at any matmul width; half the DMA
    bytes). ALiBi bias rows are hi/lo split in bf16 (4 aug rows) so logits
    keep ~16 mantissa bits.
  - ALiBi band cutoff tightened: DELTA = 8/slope (e^-8 tail) instead of 30.
  - softmax normalization reads the Y PSUM accumulator directly (no psum
    evacuation, no SBUF->SBUF denominator DMA).
  - out projection in [128,512] psum chunks (double buffered), evacuated on
    the Act engine in bf16, partial outputs summed on host in f32.

Kernel math (folded into matmuls so softmax is one exp pass):
  - scores computed TRANSPOSED (s on partitions, t free) so exp(S^T)=P^T
    lands in the lhsT layout the P@V matmul needs.
  - ALiBi bias slope*s, stability offset -(slope*t + C), and /sqrt(D) scale
    fold into 4 extra contraction rows: q' = [q/8, 1, 1, qhi, qlo],
    k' = [k, khi, klo, 1, 1] with hi+lo exact bf16 splits.
  - V gets a ones column so the softmax denominator appears as row 64 of the
    unnormalized y^T accumulator; normalization commutes with the head-dim
    contraction and is applied before the output projection.
"""

import numpy as np
import ml_dtypes

BF = ml_dtypes.bfloat16

B, T, C, H = 2, 2048, 1024, 16
D = C // H          # 64
HL = 4              # heads per core
NCORES = 8
COFF = 8.0          # softmax stability offset
# Slot h holds global heads {h*4+g : g}; flattest slope in slot h is
# 2^(-2(h+1)); keys further than DELTA[h] behind the query contribute
# < e^-8 of the softmax mass -> skip.
DELTA = [32, 128, 512, 2048]

_prog_cache = {}
DEBUG_DUMP = False


def _build_program():
    import concourse.bass as bass  # noqa: F401
    import concourse.mybir as mybir
    import concourse.tile as tile
    from concourse import bacc

    f32 = mybir.dt.float32
    bf16 = mybir.dt.bfloat16
    EXP = mybir.ActivationFunctionType.Exp
    CPY = mybir.ActivationFunctionType.Copy

    nc = bacc.Bacc("TRN2", target_bir_lowering=False, num_devices=NCORES)

    xT_in = nc.declare_dram_parameter("xT", [C, T], bf16, isOutput=False)
    wqk_in = nc.declare_dram_parameter("wqk", [4, 128, 8, 128], bf16, isOutput=False)
    wv_in = nc.declare_dram_parameter("wv", [128, 8, 256], bf16, isOutput=False)
    wp_in = nc.declare_dram_parameter("wp", [128, 2, 1024], bf16, isOutput=False)
    bqk_in = nc.declare_dram_parameter("bqk", [128, 4], f32, isOutput=False)
    bv_in = nc.declare_dram_parameter("bv", [1, 256], bf16, isOutput=False)
    # aug rows per head [HL, 64, T]: rows 60:64 = [1,1,qhi,qlo] (q side) /
    # [khi,klo,1,1] (k side); rows 0:60 zero (odd-head padding).
    augq_in = nc.declare_dram_parameter("augq", [HL, 64, T], bf16, isOutput=False)
    augk_in = nc.declare_dram_parameter("augk", [HL, 64, T], bf16, isOutput=False)
    out_dram = nc.declare_dram_parameter("out", [T, C], bf16, isOutput=True)
    if DEBUG_DUMP:
        qp_dump = nc.declare_dram_parameter("qp_dump", [HL, 128, T], bf16, isOutput=True)
        kp_dump = nc.declare_dram_parameter("kp_dump", [HL, 128, T], bf16, isOutput=True)
        vp_dump = nc.declare_dram_parameter("vp_dump", [16, 128, HL, 65], bf16, isOutput=True)
        pair_dump = nc.declare_dram_parameter("pair_dump", [2, 128, T], bf16, isOutput=True)
        y_dump = nc.declare_dram_parameter("y_dump", [HL, 4, 65, 512], f32, isOutput=True)
        rr_dump = nc.declare_dram_parameter("rr_dump", [HL, 4, 1, 512], f32, isOutput=True)
        rbc_dump = nc.declare_dram_parameter("rbc_dump", [HL, 4, 64, 512], f32, isOutput=True)

    with tile.TileContext(nc) as tc:
        with (
            tc.tile_pool(name="persist", bufs=1) as pp,
            tc.tile_pool(name="consts", bufs=1) as cp,
        ):
            p2 = tc.alloc_tile_pool(name="ph2", bufs=2)
            p2pt = tc.alloc_tile_pool(name="ph2pt", bufs=2)
            p3 = tc.alloc_tile_pool(name="ph3", bufs=2)
            ps2a = tc.alloc_tile_pool(name="ps2a", bufs=2, space="PSUM")
            ps2b = tc.alloc_tile_pool(name="ps2b", bufs=1, space="PSUM")
            psP = tc.alloc_tile_pool(name="psP", bufs=2, space="PSUM")

            # ---- prefetches: x t-super 0 first so PE starts earliest ----
            xt = [cp.tile([128, T], bf16, name=f"xt{c}", tag=f"xt{c}") for c in range(8)]
            for c in range(8):
                nc.sync.dma_start(out=xt[c][:, 0:512], in_=xT_in[128 * c:128 * (c + 1), 0:512])
            wqk_sb = [cp.tile([128, 8, 128], bf16, name=f"wqk{m}", tag=f"wqk{m}") for m in range(4)]
            for m in range(4):
                nc.sync.dma_start(out=wqk_sb[m], in_=wqk_in[m])
            wv_sb = cp.tile([128, 8, 256], bf16)
            nc.sync.dma_start(out=wv_sb, in_=wv_in[:, :, :])
            bqk_sb = cp.tile([128, 4], f32)
            nc.sync.dma_start(out=bqk_sb, in_=bqk_in[:, :])
            bv_sb = cp.tile([1, 256], bf16)
            nc.sync.dma_start(out=bv_sb, in_=bv_in[:, :])
            ones_t = cp.tile([1, 128], bf16)
            nc.vector.memset(ones_t, 1.0)

            # ---- persistent attention operands ----
            # Q'/K' per head: [128, T]. Even local head: rows 0-63 head data,
            # rows 64-67 augs. Odd local head: rows 60-63 augs, 64-127 data
            # (zero rows cost nothing: PE time ~ N only).
            QP = [pp.tile([128, T], bf16, name=f"QP{h}", tag=f"QP{h}") for h in range(HL)]
            KP = [pp.tile([128, T], bf16, name=f"KP{h}", tag=f"KP{h}") for h in range(HL)]
            # V' per s-block: [128, HL, 65] (cols 0-63 = v, col 64 = ones)
            VP = [pp.tile([128, HL, 65], bf16, name=f"VP{j}", tag=f"VP{j}") for j in range(16)]
            # normalized y^T stacked per head pair: [128, T]
            PAIR = [pp.tile([128, T], bf16, name=f"PAIR{p}", tag=f"PAIR{p}") for p in range(2)]

            for h in range(HL):
                if h % 2 == 0:
                    nc.sync.dma_start(out=QP[h][64:68, :], in_=augq_in[h, 60:64, :])
                    nc.sync.dma_start(out=KP[h][64:68, :], in_=augk_in[h, 60:64, :])
                else:
                    nc.sync.dma_start(out=QP[h][0:64, :], in_=augq_in[h, :, :])
                    nc.sync.dma_start(out=KP[h][0:64, :], in_=augk_in[h, :, :])
            # x t-supers 1..3
            for c in range(8):
                nc.sync.dma_start(out=xt[c][:, 512:T], in_=xT_in[128 * c:128 * (c + 1), 512:T])
            for j in range(16):
                nc.vector.memset(VP[j][:, :, 64:65], 1.0)

            wp_sb = cp.tile([128, 2, 1024], bf16)
            nc.sync.dma_start(out=wp_sb, in_=wp_in[:, :, :])

            # ===== interleaved pipeline: projections feed attention =====
            # PSUM budget (8 banks): psP proj staging (2) + scores (4)
            # + y accumulators (2); after psP release, psF takes its banks.
            psF = [None]

            def emit_ts(ts):
                tsl = slice(512 * ts, 512 * (ts + 1))
                if ts == 0:
                    # interleave two chains so PE tracks DMA chunk arrivals
                    for pair in ((0, 1), (2, 3)):
                        tiles = {
                            m: psP.tile([128, 512], f32, tag="p1", name=f"qk0_{m}")
                            for m in pair
                        }
                        for c in range(8):
                            for m in pair:
                                nc.tensor.matmul(
                                    tiles[m],
                                    wqk_sb[m][:, c, :],
                                    xt[c][:, tsl],
                                    start=(c == 0),
                                    stop=(c == 7),
                                )
                        for m in pair:
                            _evac_qk(tiles[m], m, tsl)
                else:
                    for m in range(4):
                        qk = psP.tile([128, 512], f32, tag="p1")
                        for c in range(8):
                            nc.tensor.matmul(
                                qk,
                                wqk_sb[m][:, c, :],
                                xt[c][:, tsl],
                                start=(c == 0),
                                stop=(c == 7),
                            )
                        _evac_qk(qk, m, tsl)
                for k in range(4):
                    jj = 4 * ts + k
                    ksl = slice(512 * ts + 128 * k, 512 * ts + 128 * (k + 1))
                    vp = psP.tile([128, 512], f32, tag="p1")
                    for c in range(8):
                        nc.tensor.matmul(
                            vp[:, 0:256],
                            xt[c][:, ksl],
                            wv_sb[:, c, :],
                            start=(c == 0),
                            stop=False,
                        )
                    nc.tensor.matmul(vp[:, 0:256], ones_t, bv_sb, start=False, stop=True)
                    nc.vector.tensor_copy(
                        VP[jj][:, :, 0:64],
                        vp[:, 0:256].rearrange("p (h d) -> p h d", h=HL),
                    )

            def _evac_qk(qk, m, tsl):
                dest = QP if m < 2 else KP
                h0 = 2 * (m % 2)
                nc.vector.tensor_scalar_add(
                    dest[h0][0:64, tsl], qk[0:64, :], bqk_sb[0:64, m:m + 1]
                )
                nc.vector.tensor_scalar_add(
                    dest[h0 + 1][64:128, tsl], qk[64:128, :], bqk_sb[64:128, m:m + 1]
                )

            def normalize(h, i, yt):
                """Divide y rows by the denominator row (64), store to PAIR."""
                tsl = slice(512 * i, 512 * (i + 1))
                ysb = p2.tile([65, 512], f32, tag="ysb")
                nc.vector.tensor_copy(ysb, yt)  # frees the psum bank fast
                if DEBUG_DUMP:
                    nc.sync.dma_start(out=y_dump[h, i], in_=ysb)
                den = p2.tile([1, 512], f32, tag="den")
                nc.sync.dma_start(out=den, in_=ysb[64:65, :])
                rr = p2.tile([1, 512], f32, tag="rr")
                nc.vector.reciprocal_approx_fast(out=rr, in_=den)
                rbc = p2.tile([64, 512], f32, tag="rbc")
                nc.gpsimd.partition_broadcast(out_ap=rbc, in_ap=rr)
                if DEBUG_DUMP:
                    nc.sync.dma_start(out=rr_dump[h, i], in_=rr)
                    nc.sync.dma_start(out=rbc_dump[h, i], in_=rbc)
                if h % 2 == 0:
                    nc.vector.tensor_mul(PAIR[h // 2][0:64, tsl], ysb[0:64, :], rbc)
                else:
                    stg = p2.tile([64, 512], bf16, tag="stg")
                    nc.vector.tensor_mul(stg, ysb[0:64, :], rbc)
                    nc.sync.dma_start(out=PAIR[h // 2][64:128, tsl], in_=stg)

            def project(i):
                """Output projection for t-blocks of t-tile i (all heads done)."""
                for tb in range(4 * i, 4 * i + 4):
                    tsl = slice(128 * tb, 128 * (tb + 1))
                    ob = p3.tile([128, 1024], bf16, tag="ob")
                    for n in range(2):
                        nsl = slice(512 * n, 512 * (n + 1))
                        fp = psF[0].tile([128, 512], f32, tag="fp")
                        for p in range(2):
                            nc.tensor.matmul(
                                fp,
                                PAIR[p][:, tsl],
                                wp_sb[:, p, nsl],
                                start=(p == 0),
                                stop=(p == 1),
                            )
                        nc.scalar.activation(ob[:, nsl], fp, CPY)
                    nc.sync.dma_start(out=out_dram[tsl, :], in_=ob)

            def emit_att(th, hs, proj_after=()):
                tbase = 1024 * th
                ilo_half, ihi_half = 2 * th, 2 * th + 2
                for h in hs:
                    rows = slice(0, 68) if h % 2 == 0 else slice(0, 128)
                    Y = {}
                    started = set()
                    for j in range(8 * th + 8):
                        i0, m = j // 4, j % 4
                        off = 128 * m
                        ilo = max(i0, ilo_half)
                        kept = [
                            i for i in range(ilo, ihi_half)
                            if 128 * j + 127 >= 512 * i - DELTA[h]
                        ]
                        if not kept:
                            continue
                        imax = kept[-1]
                        S = ps2a.tile([128, 1024], f32, tag="sc")
                        for i in kept:
                            a = 512 * i - tbase + (off if i == i0 else 0)
                            b = 512 * i - tbase + 512
                            nc.tensor.matmul(
                                S[:, a:b],
                                KP[h][rows, 128 * j:128 * (j + 1)],
                                QP[h][rows, tbase + a:tbase + b],
                                start=True,
                                stop=True,
                            )
                        amin = 512 * kept[0] - tbase + (off if kept[0] == i0 else 0)
                        amax = 512 * imax - tbase + 512
                        PT = p2pt.tile([128, 1024], bf16, tag="pt")
                        nc.scalar.activation(PT[:, amin:amax], S[:, amin:amax], EXP)
                        if i0 >= ilo_half:
                            d0 = 512 * i0 - tbase + off
                            nc.gpsimd.affine_select(
                                out=PT[:, d0:d0 + 128],
                                in_=PT[:, d0:d0 + 128],
                                compare_op=mybir.AluOpType.is_ge,
                                fill=0.0,
                                base=0,
                                pattern=[[1, 128]],
                                channel_multiplier=-1,
                            )
                        for i in sorted(kept, reverse=True):
                            if i not in Y:
                                yt = ps2b.tile(
                                    [65, 512], f32,
                                    tag=f"yb{i % 2}", name=f"Y{h}_{i}",
                                )
                                Y[i] = yt
                            a = 512 * i - tbase + (off if i == i0 else 0)
                            b = 512 * i - tbase + 512
                            ya = a - (512 * i - tbase)
                            nc.tensor.matmul(
                                Y[i][:, ya:512],
                                VP[j][:, h, :],
                                PT[:, a:b],
                                start=(i not in started),
                                stop=(j == 4 * i + 3),
                            )
                            started.add(i)
                        if j >= 3 and (j - 3) % 4 == 0:
                            i_done = (j - 3) // 4
                            if ilo_half <= i_done < ihi_half:
                                normalize(h, i_done, Y[i_done])
                                if h == hs[-1] and i_done in proj_after:
                                    project(i_done)

            # --- interleaved emission ---
            emit_ts(0)
            emit_ts(1)
            emit_att(0, [0, 1])
            emit_ts(2)
            emit_att(0, [2, 3])
            emit_ts(3)
            psP.release()
            psF[0] = tc.alloc_tile_pool(name="psF", bufs=2, space="PSUM")
            project(0)
            project(1)
            # hs order: odd slots (1,3) first so their PAIR DMA shifts land
            # early; even slot 2 last so the tail has no DMA dependency.
            emit_att(1, [1, 3, 0, 2], proj_after=(2, 3))
            if DEBUG_DUMP:
                for h in range(HL):
                    nc.sync.dma_start(out=qp_dump[h], in_=QP[h][:, :])
                    nc.sync.dma_start(out=kp_dump[h], in_=KP[h][:, :])
                for j in range(16):
                    nc.sync.dma_start(out=vp_dump[j], in_=VP[j][:, :, :])
                for p in range(2):
                    nc.sync.dma_start(out=pair_dump[p], in_=PAIR[p][:, :])
            psF[0].release()
            ps2b.release()
            ps2a.release()
            p3.release()
            p2pt.release()
            p2.release()

    nc.finalize()
    return nc


def _get_program():
    if "nc" not in _prog_cache:
        _prog_cache["nc"] = _build_program()
    return _prog_cache["nc"]


def _bf(a):
    return np.asarray(a, np.float32).astype(BF)


def _prep_core_inputs(core, x, w_attn, b_attn, w_proj):
    b, g = core // 4, core % 4
    # slot i holds global head g + 4*i (slopes grouped by magnitude per slot)
    heads = [g + 4 * i for i in range(HL)]
    qc = [slice((0 * H + h) * D, (0 * H + h) * D + D) for h in heads]
    kc = [slice((1 * H + h) * D, (1 * H + h) * D + D) for h in heads]
    vc = [slice((2 * H + h) * D, (2 * H + h) * D + D) for h in heads]

    wq = np.concatenate([w_attn[:, s] for s in qc], 1) * 0.125
    wk = np.concatenate([w_attn[:, s] for s in kc], 1)
    wqk = np.concatenate([wq, wk], 1).astype(np.float32)          # [C, 512]
    # [C, 512] -> [m, p, c, n] where row = c*128+p, col = m*128+n
    wqk_m = wqk.reshape(8, 128, 4, 128).transpose(2, 1, 0, 3)
    wv = np.concatenate([w_attn[:, s] for s in vc], 1).astype(np.float32)
    wv_p = wv.reshape(8, 128, 256).transpose(1, 0, 2)             # [128, 8, 256]
    bq = np.concatenate([b_attn[s] for s in qc]) * 0.125
    bk = np.concatenate([b_attn[s] for s in kc])
    bqk = np.concatenate([bq, bk]).astype(np.float32).reshape(4, 128).T.copy()
    bv = np.concatenate([b_attn[s] for s in vc]).astype(np.float32)[None, :]
    wp = np.concatenate([w_proj[s, :] for s in qc], 0).astype(np.float32)  # [256, C]
    wp_p = wp.reshape(2, 128, 1024).transpose(1, 0, 2)            # [128, 2, 1024]

    slopes = 2.0 ** (-(8.0 / H) * (np.array(heads, np.float64) + 1.0))
    pos = np.arange(T, dtype=np.float64)
    kaug = slopes[:, None] * pos[None, :]                          # [HL, T]
    khi = _bf(kaug)
    klo = _bf(kaug - khi.astype(np.float64))
    qaug = -(kaug + COFF)
    qhi = _bf(qaug)
    qlo = _bf(qaug - qhi.astype(np.float64))

    augq = np.zeros((HL, 64, T), BF)
    augq[:, 60, :] = BF(1.0)
    augq[:, 61, :] = BF(1.0)
    augq[:, 62, :] = qhi
    augq[:, 63, :] = qlo
    augk = np.zeros((HL, 64, T), BF)
    augk[:, 60, :] = khi
    augk[:, 61, :] = klo
    augk[:, 62, :] = BF(1.0)
    augk[:, 63, :] = BF(1.0)

    return {
        "xT": _bf(np.ascontiguousarray(x[b].T)),
        "wqk": _bf(np.ascontiguousarray(wqk_m)),
        "wv": _bf(np.ascontiguousarray(wv_p)),
        "wp": _bf(np.ascontiguousarray(wp_p)),
        "bqk": bqk,
        "bv": _bf(bv),
        "augq": augq,
        "augk": augk,
    }


def kernel(x, w_attn, b_attn, w_proj, b_proj, _run_kwargs=None):
    from concourse.bass_utils import run_bass_kernel_spmd

    x = np.asarray(x, np.float32)
    w_attn = np.asarray(w_attn, np.float32)
    b_attn = np.asarray(b_attn, np.float32)
    w_proj = np.asarray(w_proj, np.float32)
    b_proj = np.asarray(b_proj, np.float32)

    nc = _get_program()
    in_maps = [_prep_core_inputs(c, x, w_attn, b_attn, w_proj) for c in range(NCORES)]
    res = run_bass_kernel_spmd(
        nc, in_maps, core_ids=list(range(NCORES)), **(_run_kwargs or {})
    )
    _prog_cache["last_result"] = res

    out = np.zeros((B, T, C), np.float32)
    for c in range(NCORES):
        out[c // 4] += np.asarray(res.results[c]["out"], np.float32)
    out += b_proj[None, None, :]
    return out


# revision 22
# speedup vs baseline: 1.2181x; 1.0215x over previous
"""Causal self-attention with ALiBi, sharded over 8 TRN2 NeuronCores.

Sharding: core c -> batch b = c//4, head group g = c%4 (4 heads each).
Each core computes QKV projection for its heads, causal attention, and the
partial output projection (w_proj rows of its heads). Host sums the 4
partials per batch and adds b_proj.

v2 (vs baseline):
  - x is transposed on the host: no on-chip PE transposes or staging copies.
  - everything runs bf16 (1.0 cycle/row at any matmul width; half the DMA
    bytes). ALiBi bias rows are hi/lo split in bf16 (4 aug rows) so logits
    keep ~16 mantissa bits.
  - ALiBi band cutoff tightened: DELTA = 8/slope (e^-8 tail) instead of 30.
  - softmax normalization reads the Y PSUM accumulator directly (no psum
    evacuation, no SBUF->SBUF denominator DMA).
  - out projection in [128,512] psum chunks (double buffered), evacuated on
    the Act engine in bf16, partial outputs summed on host in f32.

Kernel math (folded into matmuls so softmax is one exp pass):
  - scores computed TRANSPOSED (s on partitions, t free) so exp(S^T)=P^T
    lands in the lhsT layout the P@V matmul needs.
  - ALiBi bias slope*s, stability offset -(slope*t + C), and /sqrt(D) scale
    fold into 4 extra contraction rows: q' = [q/8, 1, 1, qhi, qlo],
    k' = [k, khi, klo, 1, 1] with hi+lo exact bf16 splits.
  - V gets a ones column so the softmax denominator appears as row 64 of the
    unnormalized y^T accumulator; normalization commutes with the head-dim
    contraction and is applied before the output projection.
"""

from collections import deque

import numpy as np
import ml_dtypes

BF = ml_dtypes.bfloat16

B, T, C, H = 2, 2048, 1024, 16
D = C // H          # 64
HL = 4              # heads per core
NCORES = 8
COFF = 8.0          # softmax stability offset
# Slot h holds global heads {h*4+g : g}; flattest slope in slot h is
# 2^(-2(h+1)); keys further than DELTA[h] behind the query contribute
# < e^-8 of the softmax mass -> skip.
DELTA = [32, 128, 512, 2048]

_prog_cache = {}
DEBUG_DUMP = False


def _build_program():
    import concourse.bass as bass  # noqa: F401
    import concourse.mybir as mybir
    import concourse.tile as tile
    from concourse import bacc

    f32 = mybir.dt.float32
    bf16 = mybir.dt.bfloat16
    EXP = mybir.ActivationFunctionType.Exp
    CPY = mybir.ActivationFunctionType.Copy

    nc = bacc.Bacc("TRN2", target_bir_lowering=False, num_devices=NCORES)

    xT_in = nc.declare_dram_parameter("xT", [C, T], bf16, isOutput=False)
    wqk_in = nc.declare_dram_parameter("wqk", [4, 128, 8, 128], bf16, isOutput=False)
    wv_in = nc.declare_dram_parameter("wv", [128, 8, 256], bf16, isOutput=False)
    wp_in = nc.declare_dram_parameter("wp", [128, 2, 1024], bf16, isOutput=False)
    bqk_in = nc.declare_dram_parameter("bqk", [128, 4], f32, isOutput=False)
    bv_in = nc.declare_dram_parameter("bv", [1, 256], bf16, isOutput=False)
    # aug rows per head [HL, 64, T]: rows 60:64 = [1,1,qhi,qlo] (q side) /
    # [khi,klo,1,1] (k side); rows 0:60 zero (odd-head padding).
    augq_in = nc.declare_dram_parameter("augq", [HL, 64, T], bf16, isOutput=False)
    augk_in = nc.declare_dram_parameter("augk", [HL, 64, T], bf16, isOutput=False)
    out_dram = nc.declare_dram_parameter("out", [T, C], bf16, isOutput=True)
    if DEBUG_DUMP:
        qp_dump = nc.declare_dram_parameter("qp_dump", [HL, 128, T], bf16, isOutput=True)
        kp_dump = nc.declare_dram_parameter("kp_dump", [HL, 128, T], bf16, isOutput=True)
        vp_dump = nc.declare_dram_parameter("vp_dump", [16, 128, HL, 65], bf16, isOutput=True)
        pair_dump = nc.declare_dram_parameter("pair_dump", [2, 128, T], bf16, isOutput=True)
        y_dump = nc.declare_dram_parameter("y_dump", [HL, 4, 65, 512], f32, isOutput=True)
        rr_dump = nc.declare_dram_parameter("rr_dump", [HL, 4, 1, 512], f32, isOutput=True)
        rbc_dump = nc.declare_dram_parameter("rbc_dump", [HL, 4, 64, 512], f32, isOutput=True)

    with tile.TileContext(nc) as tc:
        with (
            tc.tile_pool(name="persist", bufs=1) as pp,
            tc.tile_pool(name="consts", bufs=1) as cp,
        ):
            p2 = tc.alloc_tile_pool(name="ph2", bufs=2)
            p2pt = tc.alloc_tile_pool(name="ph2pt", bufs=2)
            p3 = tc.alloc_tile_pool(name="ph3", bufs=2)
            ps2a = tc.alloc_tile_pool(name="ps2a", bufs=2, space="PSUM")
            ps2b = tc.alloc_tile_pool(name="ps2b", bufs=1, space="PSUM")
            psP = tc.alloc_tile_pool(name="psP", bufs=2, space="PSUM")

            # ---- prefetches: wqk m0/m1 then x t-super 0 chunks, so the
            # first interleaved qk chains start as soon as chunks land ----
            wqk_sb = [cp.tile([128, 8, 128], bf16, name=f"wqk{m}", tag=f"wqk{m}") for m in range(4)]
            for m in range(2):
                nc.sync.dma_start(out=wqk_sb[m], in_=wqk_in[m])
            xt = [cp.tile([128, T], bf16, name=f"xt{c}", tag=f"xt{c}") for c in range(8)]
            for c in range(8):
                nc.sync.dma_start(out=xt[c][:, 0:512], in_=xT_in[128 * c:128 * (c + 1), 0:512])
            for m in range(2, 4):
                nc.sync.dma_start(out=wqk_sb[m], in_=wqk_in[m])
            wv_sb = cp.tile([128, 8, 256], bf16)
            nc.sync.dma_start(out=wv_sb, in_=wv_in[:, :, :])
            bqk_sb = cp.tile([128, 4], f32)
            nc.sync.dma_start(out=bqk_sb, in_=bqk_in[:, :])
            bv_sb = cp.tile([1, 256], bf16)
            nc.sync.dma_start(out=bv_sb, in_=bv_in[:, :])
            ones_t = cp.tile([1, 128], bf16)
            nc.vector.memset(ones_t, 1.0)

            # ---- persistent attention operands ----
            # Q'/K' per head: [128, T]. Even local head: rows 0-63 head data,
            # rows 64-67 augs. Odd local head: rows 60-63 augs, 64-127 data
            # (zero rows cost nothing: PE time ~ N only).
            QP = [pp.tile([128, T], bf16, name=f"QP{h}", tag=f"QP{h}") for h in range(HL)]
            KP = [pp.tile([128, T], bf16, name=f"KP{h}", tag=f"KP{h}") for h in range(HL)]
            # V' per s-block: [128, HL, 65] (cols 0-63 = v, col 64 = ones)
            VP = [pp.tile([128, HL, 65], bf16, name=f"VP{j}", tag=f"VP{j}") for j in range(16)]
            # normalized y^T stacked per head pair: [128, T]
            PAIR = [pp.tile([128, T], bf16, name=f"PAIR{p}", tag=f"PAIR{p}") for p in range(2)]

            for h in range(HL):
                if h % 2 == 0:
                    nc.sync.dma_start(out=QP[h][64:68, :], in_=augq_in[h, 60:64, :])
                    nc.sync.dma_start(out=KP[h][64:68, :], in_=augk_in[h, 60:64, :])
                else:
                    nc.sync.dma_start(out=QP[h][0:64, :], in_=augq_in[h, :, :])
                    nc.sync.dma_start(out=KP[h][0:64, :], in_=augk_in[h, :, :])
            # x t-supers 1..3
            for c in range(8):
                nc.sync.dma_start(out=xt[c][:, 512:T], in_=xT_in[128 * c:128 * (c + 1), 512:T])
            for j in range(16):
                nc.vector.memset(VP[j][:, :, 64:65], 1.0)

            wp_sb = cp.tile([128, 2, 1024], bf16)
            nc.sync.dma_start(out=wp_sb, in_=wp_in[:, :, :])

            # ===== interleaved pipeline: projections feed attention =====
            # PSUM budget (8 banks): psP proj staging (2) + scores (4)
            # + y accumulators (2); after psP release, psF takes its banks.
            psF = [None]

            def emit_ts(ts):
                tsl = slice(512 * ts, 512 * (ts + 1))
                if ts == 0:
                    # interleave two chains so PE tracks DMA chunk arrivals
                    for pair in ((0, 1), (2, 3)):
                        tiles = {
                            m: psP.tile([128, 512], f32, tag="p1", name=f"qk0_{m}")
                            for m in pair
                        }
                        for c in range(8):
                            for m in pair:
                                nc.tensor.matmul(
                                    tiles[m],
                                    wqk_sb[m][:, c, :],
                                    xt[c][:, tsl],
                                    start=(c == 0),
                                    stop=(c == 7),
                                )
                        for m in pair:
                            _evac_qk(tiles[m], m, tsl)
                else:
                    for m in range(4):
                        qk = psP.tile([128, 512], f32, tag="p1")
                        for c in range(8):
                            nc.tensor.matmul(
                                qk,
                                wqk_sb[m][:, c, :],
                                xt[c][:, tsl],
                                start=(c == 0),
                                stop=(c == 7),
                            )
                        _evac_qk(qk, m, tsl)
                for k in range(4):
                    jj = 4 * ts + k
                    ksl = slice(512 * ts + 128 * k, 512 * ts + 128 * (k + 1))
                    vp = psP.tile([128, 512], f32, tag="p1")
                    for c in range(8):
                        nc.tensor.matmul(
                            vp[:, 0:256],
                            xt[c][:, ksl],
                            wv_sb[:, c, :],
                            start=(c == 0),
                            stop=False,
                        )
                    nc.tensor.matmul(vp[:, 0:256], ones_t, bv_sb, start=False, stop=True)
                    nc.vector.tensor_copy(
                        VP[jj][:, :, 0:64],
                        vp[:, 0:256].rearrange("p (h d) -> p h d", h=HL),
                    )

            def _evac_qk(qk, m, tsl):
                dest = QP if m < 2 else KP
                h0 = 2 * (m % 2)
                nc.vector.tensor_scalar_add(
                    dest[h0][0:64, tsl], qk[0:64, :], bqk_sb[0:64, m:m + 1]
                )
                nc.vector.tensor_scalar_add(
                    dest[h0 + 1][64:128, tsl], qk[64:128, :], bqk_sb[64:128, m:m + 1]
                )

            def normalize(h, i, yt):
                """Divide y rows by the denominator row (64), store to PAIR.

                DVE tensor ops handle PSUM sources and partition-shifted
                outputs, so this reads the Y accumulator directly; only the
                custom-DVE recip and the gpsimd broadcast need partition-0
                inputs (hence the den shift-copy)."""
                tsl = slice(512 * i, 512 * (i + 1))
                if DEBUG_DUMP:
                    nc.sync.dma_start(out=y_dump[h, i], in_=yt)
                den = p2.tile([1, 512], f32, tag="den")
                nc.vector.tensor_copy(den, yt[64:65, :])
                rr = p2.tile([1, 512], f32, tag="rr")
                nc.vector.reciprocal_approx_fast(out=rr, in_=den)
                rbc = p2.tile([64, 512], f32, tag="rbc")
                nc.gpsimd.partition_broadcast(out_ap=rbc, in_ap=rr)
                if DEBUG_DUMP:
                    nc.sync.dma_start(out=rr_dump[h, i], in_=rr)
                    nc.sync.dma_start(out=rbc_dump[h, i], in_=rbc)
                rows = slice(0, 64) if h % 2 == 0 else slice(64, 128)
                nc.vector.tensor_mul(PAIR[h // 2][rows, tsl], yt[0:64, :], rbc)

            def proj_tb(tb):
                """Output projection for one 128-row t-block (PAIR[1] first:
                its slots normalize earlier in the att(1) hs order)."""
                tsl = slice(128 * tb, 128 * (tb + 1))
                ob = p3.tile([128, 1024], bf16, tag="ob")
                for n in range(2):
                    nsl = slice(512 * n, 512 * (n + 1))
                    fp = psF[0].tile([128, 512], f32, tag="fp")
                    for p in (1, 0):
                        nc.tensor.matmul(
                            fp,
                            PAIR[p][:, tsl],
                            wp_sb[:, p, nsl],
                            start=(p == 1),
                            stop=(p == 0),
                        )
                    nc.scalar.activation(ob[:, nsl], fp, CPY)
                nc.sync.dma_start(out=out_dram[tsl, :], in_=ob)

            def proj_chunks(i):
                return [
                    (lambda tb=tb: proj_tb(tb)) for tb in range(4 * i, 4 * i + 4)
                ]

            def project(i):
                for f in proj_chunks(i):
                    f()

            def emit_att(th, hs, proj_after=(), fillq=None):
                tbase = 1024 * th
                ilo_half, ihi_half = 2 * th, 2 * th + 2
                for h in hs:
                    rows = slice(0, 68) if h % 2 == 0 else slice(0, 128)
                    Y = {}
                    started = set()
                    for j in range(8 * th + 8):
                        i0, m = j // 4, j % 4
                        off = 128 * m
                        ilo = max(i0, ilo_half)
                        kept = [
                            i for i in range(ilo, ihi_half)
                            if 128 * j + 127 >= 512 * i - DELTA[h]
                        ]
                        if not kept:
                            continue
                        imax = kept[-1]
                        S = ps2a.tile([128, 1024], f32, tag="sc")
                        for i in kept:
                            a = 512 * i - tbase + (off if i == i0 else 0)
                            b = 512 * i - tbase + 512
                            nc.tensor.matmul(
                                S[:, a:b],
                                KP[h][rows, 128 * j:128 * (j + 1)],
                                QP[h][rows, tbase + a:tbase + b],
                                start=True,
                                stop=True,
                            )
                        amin = 512 * kept[0] - tbase + (off if kept[0] == i0 else 0)
                        amax = 512 * imax - tbase + 512
                        PT = p2pt.tile([128, 1024], bf16, tag="pt")
                        nc.scalar.activation(PT[:, amin:amax], S[:, amin:amax], EXP)
                        if i0 >= ilo_half:
                            d0 = 512 * i0 - tbase + off
                            nc.gpsimd.affine_select(
                                out=PT[:, d0:d0 + 128],
                                in_=PT[:, d0:d0 + 128],
                                compare_op=mybir.AluOpType.is_ge,
                                fill=0.0,
                                base=0,
                                pattern=[[1, 128]],
                                channel_multiplier=-1,
                            )
                        for i in sorted(kept, reverse=True):
                            if i not in Y:
                                yt = ps2b.tile(
                                    [65, 512], f32,
                                    tag=f"yb{i % 2}", name=f"Y{h}_{i}",
                                )
                                Y[i] = yt
                            a = 512 * i - tbase + (off if i == i0 else 0)
                            b = 512 * i - tbase + 512
                            ya = a - (512 * i - tbase)
                            nc.tensor.matmul(
                                Y[i][:, ya:512],
                                VP[j][:, h, :],
                                PT[:, a:b],
                                start=(i not in started),
                                stop=(j == 4 * i + 3),
                            )
                            started.add(i)
                        if j >= 3 and (j - 3) % 4 == 0:
                            i_done = (j - 3) // 4
                            if ilo_half <= i_done < ihi_half:
                                normalize(h, i_done, Y[i_done])
                                if h == hs[-1] and i_done in proj_after:
                                    if fillq is not None:
                                        fillq.extend(proj_chunks(i_done))
                                    else:
                                        project(i_done)
                        if fillq:
                            fillq.popleft()()
                if fillq:
                    while fillq:
                        fillq.popleft()()

            # --- interleaved emission ---
            emit_ts(0)
            emit_ts(1)
            emit_att(0, [0, 1])
            emit_ts(2)
            emit_att(0, [2, 3])
            emit_ts(3)
            psP.release()
            psF[0] = tc.alloc_tile_pool(name="psF", bufs=2, space="PSUM")
            # proj work drains one t-block per attention j-step so the PE
            # always has independent work while the Act engine runs exps.
            fillq = deque()
            fillq.extend(proj_chunks(0))
            fillq.extend(proj_chunks(1))
            emit_att(1, [1, 3, 2, 0], proj_after=(2, 3), fillq=fillq)
            if DEBUG_DUMP:
                for h in range(HL):
                    nc.sync.dma_start(out=qp_dump[h], in_=QP[h][:, :])
                    nc.sync.dma_start(out=kp_dump[h], in_=KP[h][:, :])
                for j in range(16):
                    nc.sync.dma_start(out=vp_dump[j], in_=VP[j][:, :, :])
                for p in range(2):
                    nc.sync.dma_start(out=pair_dump[p], in_=PAIR[p][:, :])
            psF[0].release()
            ps2b.release()
            ps2a.release()
            p3.release()
            p2pt.release()
            p2.release()

    nc.finalize()
    return nc


def _get_program():
    if "nc" not in _prog_cache:
        _prog_cache["nc"] = _build_program()
    return _prog_cache["nc"]


def _bf(a):
    return np.asarray(a, np.float32).astype(BF)


def _prep_core_inputs(core, x, w_attn, b_attn, w_proj):
    b, g = core // 4, core % 4
    # slot i holds global head g + 4*i (slopes grouped by magnitude per slot)
    heads = [g + 4 * i for i in range(HL)]
    qc = [slice((0 * H + h) * D, (0 * H + h) * D + D) for h in heads]
    kc = [slice((1 * H + h) * D, (1 * H + h) * D + D) for h in heads]
    vc = [slice((2 * H + h) * D, (2 * H + h) * D + D) for h in heads]

    wq = np.concatenate([w_attn[:, s] for s in qc], 1) * 0.125
    wk = np.concatenate([w_attn[:, s] for s in kc], 1)
    wqk = np.concatenate([wq, wk], 1).astype(np.float32)          # [C, 512]
    # [C, 512] -> [m, p, c, n] where row = c*128+p, col = m*128+n
    wqk_m = wqk.reshape(8, 128, 4, 128).transpose(2, 1, 0, 3)
    wv = np.concatenate([w_attn[:, s] for s in vc], 1).astype(np.float32)
    wv_p = wv.reshape(8, 128, 256).transpose(1, 0, 2)             # [128, 8, 256]
    bq = np.concatenate([b_attn[s] for s in qc]) * 0.125
    bk = np.concatenate([b_attn[s] for s in kc])
    bqk = np.concatenate([bq, bk]).astype(np.float32).reshape(4, 128).T.copy()
    bv = np.concatenate([b_attn[s] for s in vc]).astype(np.float32)[None, :]
    wp = np.concatenate([w_proj[s, :] for s in qc], 0).astype(np.float32)  # [256, C]
    wp_p = wp.reshape(2, 128, 1024).transpose(1, 0, 2)            # [128, 2, 1024]

    slopes = 2.0 ** (-(8.0 / H) * (np.array(heads, np.float64) + 1.0))
    pos = np.arange(T, dtype=np.float64)
    kaug = slopes[:, None] * pos[None, :]                          # [HL, T]
    khi = _bf(kaug)
    klo = _bf(kaug - khi.astype(np.float64))
    qaug = -(kaug + COFF)
    qhi = _bf(qaug)
    qlo = _bf(qaug - qhi.astype(np.float64))

    augq = np.zeros((HL, 64, T), BF)
    augq[:, 60, :] = BF(1.0)
    augq[:, 61, :] = BF(1.0)
    augq[:, 62, :] = qhi
    augq[:, 63, :] = qlo
    augk = np.zeros((HL, 64, T), BF)
    augk[:, 60, :] = khi
    augk[:, 61, :] = klo
    augk[:, 62, :] = BF(1.0)
    augk[:, 63, :] = BF(1.0)

    return {
        "xT": _bf(np.ascontiguousarray(x[b].T)),
        "wqk": _bf(np.ascontiguousarray(wqk_m)),
        "wv": _bf(np.ascontiguousarray(wv_p)),
        "wp": _bf(np.ascontiguousarray(wp_p)),
        "bqk": bqk,
        "bv": _bf(bv),
        "augq": augq,
        "augk": augk,
    }


def kernel(x, w_attn, b_attn, w_proj, b_proj, _run_kwargs=None):
    from concourse.bass_utils import run_bass_kernel_spmd

    x = np.asarray(x, np.float32)
    w_attn = np.asarray(w_attn, np.float32)
    b_attn = np.asarray(b_attn, np.float32)
    w_proj = np.asarray(w_proj, np.float32)
    b_proj = np.asarray(b_proj, np.float32)

    nc = _get_program()
    in_maps = [_prep_core_inputs(c, x, w_attn, b_attn, w_proj) for c in range(NCORES)]
    res = run_bass_kernel_spmd(
        nc, in_maps, core_ids=list(range(NCORES)), **(_run_kwargs or {})
    )
    _prog_cache["last_result"] = res

    out = np.zeros((B, T, C), np.float32)
    for c in range(NCORES):
        out[c // 4] += np.asarray(res.results[c]["out"], np.float32)
    out += b_proj[None, None, :]
    return out


# revision 26
# speedup vs baseline: 1.3397x; 1.0999x over previous
"""Causal self-attention with ALiBi, sharded over 8 TRN2 NeuronCores.

Sharding: core c -> batch b = c//4, head group g = c%4 (4 heads each).
Each core computes QKV projection for its heads, causal attention, and the
partial output projection (w_proj rows of its heads). Host sums the 4
partials per batch and adds b_proj.

v2 (vs baseline):
  - x is transposed on the host: no on-chip PE transposes or staging copies.
  - everything runs bf16 (1.0 cycle/row at any matmul width; half the DMA
    bytes). ALiBi bias rows are hi/lo split in bf16 (4 aug rows) so logits
    keep ~16 mantissa bits.
  - ALiBi band cutoff tightened: DELTA = 8/slope (e^-8 tail) instead of 30.
  - softmax normalization reads the Y PSUM accumulator directly (no psum
    evacuation, no SBUF->SBUF denominator DMA).
  - out projection in [128,512] psum chunks (double buffered), evacuated on
    the Act engine in bf16, partial outputs summed on host in f32.

Kernel math (folded into matmuls so softmax is one exp pass):
  - scores computed TRANSPOSED (s on partitions, t free) so exp(S^T)=P^T
    lands in the lhsT layout the P@V matmul needs.
  - ALiBi bias slope*s, stability offset -(slope*t + C), and /sqrt(D) scale
    fold into 4 extra contraction rows: q' = [q/8, 1, 1, qhi, qlo],
    k' = [k, khi, klo, 1, 1] with hi+lo exact bf16 splits.
  - V gets a ones column so the softmax denominator appears as row 64 of the
    unnormalized y^T accumulator; normalization commutes with the head-dim
    contraction and is applied before the output projection.
"""

from collections import deque

import numpy as np
import ml_dtypes

BF = ml_dtypes.bfloat16

B, T, C, H = 2, 2048, 1024, 16
D = C // H          # 64
HL = 4              # heads per core
NCORES = 8
COFF = 8.0          # softmax stability offset
# Slot h holds global heads {h*4+g : g}; flattest slope in slot h is
# 2^(-2(h+1)); keys further than DELTA[h] behind the query contribute
# < e^-8 of the softmax mass -> skip.
DELTA = [32, 128, 512, 2048]

_prog_cache = {}
DEBUG_DUMP = False


def _build_program():
    import concourse.bass as bass  # noqa: F401
    import concourse.mybir as mybir
    import concourse.tile as tile
    from concourse import bacc

    f32 = mybir.dt.float32
    bf16 = mybir.dt.bfloat16
    EXP = mybir.ActivationFunctionType.Exp
    CPY = mybir.ActivationFunctionType.Copy

    nc = bacc.Bacc("TRN2", target_bir_lowering=False, num_devices=NCORES)

    xT_in = nc.declare_dram_parameter("xT", [C, T], bf16, isOutput=False)
    wqk_in = nc.declare_dram_parameter("wqk", [4, 128, 8, 128], bf16, isOutput=False)
    wv_in = nc.declare_dram_parameter("wv", [128, 8, 256], bf16, isOutput=False)
    wp_in = nc.declare_dram_parameter("wp", [128, 2, 1024], bf16, isOutput=False)
    bqk_in = nc.declare_dram_parameter("bqk", [128, 4], f32, isOutput=False)
    bv_in = nc.declare_dram_parameter("bv", [1, 256], bf16, isOutput=False)
    # aug rows per head [HL, 64, T]: rows 60:64 = [1,1,qhi,qlo] (q side) /
    # [khi,klo,1,1] (k side); rows 0:60 zero (odd-head padding).
    augq_in = nc.declare_dram_parameter("augq", [HL, 64, T], bf16, isOutput=False)
    augk_in = nc.declare_dram_parameter("augk", [HL, 64, T], bf16, isOutput=False)
    out_dram = nc.declare_dram_parameter("out", [T, C], bf16, isOutput=True)
    if DEBUG_DUMP:
        qp_dump = nc.declare_dram_parameter("qp_dump", [HL, 128, T], bf16, isOutput=True)
        kp_dump = nc.declare_dram_parameter("kp_dump", [HL, 128, T], bf16, isOutput=True)
        vp_dump = nc.declare_dram_parameter("vp_dump", [16, 128, HL, 65], bf16, isOutput=True)
        pair_dump = nc.declare_dram_parameter("pair_dump", [2, 128, T], bf16, isOutput=True)
        y_dump = nc.declare_dram_parameter("y_dump", [HL, 4, 65, 512], f32, isOutput=True)
        rr_dump = nc.declare_dram_parameter("rr_dump", [HL, 4, 1, 512], f32, isOutput=True)
        rbc_dump = nc.declare_dram_parameter("rbc_dump", [HL, 4, 64, 512], f32, isOutput=True)

    with tile.TileContext(nc) as tc:
        with (
            tc.tile_pool(name="persist", bufs=1) as pp,
            tc.tile_pool(name="consts", bufs=1) as cp,
        ):
            p2 = tc.alloc_tile_pool(name="ph2", bufs=2)
            p2pt = tc.alloc_tile_pool(name="ph2pt", bufs=2)
            p3 = tc.alloc_tile_pool(name="ph3", bufs=4)
            ps2a = tc.alloc_tile_pool(name="ps2a", bufs=2, space="PSUM")
            ps2b = tc.alloc_tile_pool(name="ps2b", bufs=1, space="PSUM")
            psP = tc.alloc_tile_pool(name="psP", bufs=2, space="PSUM")

            # ---- prefetches: wqk m0/m1 then x t-super 0 chunks, so the
            # first interleaved qk chains start as soon as chunks land ----
            wqk_sb = [cp.tile([128, 8, 128], bf16, name=f"wqk{m}", tag=f"wqk{m}") for m in range(4)]
            for m in range(2):
                nc.sync.dma_start(out=wqk_sb[m], in_=wqk_in[m])
            bqk_sb = cp.tile([128, 4], f32)
            nc.sync.dma_start(out=bqk_sb, in_=bqk_in[:, :])
            bv_sb = cp.tile([1, 256], bf16)
            nc.sync.dma_start(out=bv_sb, in_=bv_in[:, :])
            xt = [cp.tile([128, T], bf16, name=f"xt{c}", tag=f"xt{c}") for c in range(8)]
            for c in range(8):
                nc.sync.dma_start(out=xt[c][:, 0:512], in_=xT_in[128 * c:128 * (c + 1), 0:512])
            for m in range(2, 4):
                nc.sync.dma_start(out=wqk_sb[m], in_=wqk_in[m])
            wv_sb = cp.tile([128, 8, 256], bf16)
            nc.sync.dma_start(out=wv_sb, in_=wv_in[:, :, :])
            ones_t = cp.tile([1, 128], bf16)
            nc.vector.memset(ones_t, 1.0)

            # ---- persistent attention operands ----
            # Q'/K' per head: [128, T]. Even local head: rows 0-63 head data,
            # rows 64-67 augs. Odd local head: rows 60-63 augs, 64-127 data
            # (zero rows cost nothing: PE time ~ N only).
            QP = [pp.tile([128, T], bf16, name=f"QP{h}", tag=f"QP{h}") for h in range(HL)]
            KP = [pp.tile([128, T], bf16, name=f"KP{h}", tag=f"KP{h}") for h in range(HL)]
            # V' per s-block: [128, HL, 65] (cols 0-63 = v, col 64 = ones)
            VP = [pp.tile([128, HL, 65], bf16, name=f"VP{j}", tag=f"VP{j}") for j in range(16)]
            # normalized y^T stacked per head pair: [128, T]
            PAIR = [pp.tile([128, T], bf16, name=f"PAIR{p}", tag=f"PAIR{p}") for p in range(2)]

            # x t-super 1 first (needed by emit_ts(1) at ~PE t+14us), then
            # augs (needed by att(0) at ~25us), then ts2/ts3.
            for c in range(8):
                nc.sync.dma_start(out=xt[c][:, 512:1024], in_=xT_in[128 * c:128 * (c + 1), 512:1024])
            for h in range(HL):
                if h % 2 == 0:
                    nc.sync.dma_start(out=QP[h][64:68, :], in_=augq_in[h, 60:64, :])
                    nc.sync.dma_start(out=KP[h][64:68, :], in_=augk_in[h, 60:64, :])
                else:
                    nc.sync.dma_start(out=QP[h][0:64, :], in_=augq_in[h, :, :])
                    nc.sync.dma_start(out=KP[h][0:64, :], in_=augk_in[h, :, :])
            for c in range(8):
                nc.sync.dma_start(out=xt[c][:, 1024:T], in_=xT_in[128 * c:128 * (c + 1), 1024:T])
            for j in range(16):
                nc.vector.memset(VP[j][:, :, 64:65], 1.0)

            wp_sb = cp.tile([128, 2, 1024], bf16)
            nc.sync.dma_start(out=wp_sb, in_=wp_in[:, :, :])

            # ===== interleaved pipeline: projections feed attention =====
            # PSUM budget (8 banks): psP proj staging (2) + scores (4)
            # + y accumulators (2); after psP release, psF takes its banks.
            psF = [None]

            def emit_ts(ts):
                tsl = slice(512 * ts, 512 * (ts + 1))
                if ts == 0:
                    # interleave two chains so PE tracks DMA chunk arrivals
                    for pair in ((0, 1), (2, 3)):
                        tiles = {
                            m: psP.tile([128, 512], f32, tag="p1", name=f"qk0_{m}")
                            for m in pair
                        }
                        for c in range(8):
                            for m in pair:
                                nc.tensor.matmul(
                                    tiles[m],
                                    wqk_sb[m][:, c, :],
                                    xt[c][:, tsl],
                                    start=(c == 0),
                                    stop=(c == 7),
                                )
                        for m in pair:
                            _evac_qk(tiles[m], m, tsl)
                else:
                    for m in range(4):
                        qk = psP.tile([128, 512], f32, tag="p1")
                        for c in range(8):
                            nc.tensor.matmul(
                                qk,
                                wqk_sb[m][:, c, :],
                                xt[c][:, tsl],
                                start=(c == 0),
                                stop=(c == 7),
                            )
                        _evac_qk(qk, m, tsl)
                for k in range(4):
                    jj = 4 * ts + k
                    ksl = slice(512 * ts + 128 * k, 512 * ts + 128 * (k + 1))
                    vp = psP.tile([128, 512], f32, tag="p1")
                    for c in range(8):
                        nc.tensor.matmul(
                            vp[:, 0:256],
                            xt[c][:, ksl],
                            wv_sb[:, c, :],
                            start=(c == 0),
                            stop=False,
                        )
                    nc.tensor.matmul(vp[:, 0:256], ones_t, bv_sb, start=False, stop=True)
                    nc.vector.tensor_copy(
                        VP[jj][:, :, 0:64],
                        vp[:, 0:256].rearrange("p (h d) -> p h d", h=HL),
                    )

            def _evac_qk(qk, m, tsl):
                dest = QP if m < 2 else KP
                h0 = 2 * (m % 2)
                nc.vector.tensor_scalar_add(
                    dest[h0][0:64, tsl], qk[0:64, :], bqk_sb[0:64, m:m + 1]
                )
                nc.vector.tensor_scalar_add(
                    dest[h0 + 1][64:128, tsl], qk[64:128, :], bqk_sb[64:128, m:m + 1]
                )

            def normalize(h, i, yt):
                """Divide y rows by the denominator row (64), store to PAIR.

                DVE tensor ops handle PSUM sources and partition-shifted
                outputs, so this reads the Y accumulator directly; only the
                custom-DVE recip and the gpsimd broadcast need partition-0
                inputs (hence the den shift-copy)."""
                tsl = slice(512 * i, 512 * (i + 1))
                if DEBUG_DUMP:
                    nc.sync.dma_start(out=y_dump[h, i], in_=yt)
                den = p2.tile([1, 512], f32, tag="den")
                nc.vector.tensor_copy(den, yt[64:65, :])
                rr = p2.tile([1, 512], f32, tag="rr")
                nc.vector.reciprocal_approx_fast(out=rr, in_=den)
                rbc = p2.tile([64, 512], f32, tag="rbc")
                nc.gpsimd.partition_broadcast(out_ap=rbc, in_ap=rr)
                if DEBUG_DUMP:
                    nc.sync.dma_start(out=rr_dump[h, i], in_=rr)
                    nc.sync.dma_start(out=rbc_dump[h, i], in_=rbc)
                rows = slice(0, 64) if h % 2 == 0 else slice(64, 128)
                nc.vector.tensor_mul(PAIR[h // 2][rows, tsl], yt[0:64, :], rbc)

            def proj_tb(tb):
                """Output projection for one 128-row t-block (PAIR[1] first:
                its slots normalize earlier in the att(1) hs order)."""
                tsl = slice(128 * tb, 128 * (tb + 1))
                ob = p3.tile([128, 1024], bf16, tag="ob")
                for n in range(2):
                    nsl = slice(512 * n, 512 * (n + 1))
                    fp = psF[0].tile([128, 512], f32, tag="fp")
                    for p in (1, 0):
                        nc.tensor.matmul(
                            fp,
                            PAIR[p][:, tsl],
                            wp_sb[:, p, nsl],
                            start=(p == 1),
                            stop=(p == 0),
                        )
                    # split evacuation across engines to balance load
                    if n == 0:
                        nc.vector.tensor_copy(ob[:, nsl], fp)
                    else:
                        nc.scalar.activation(ob[:, nsl], fp, CPY)
                nc.sync.dma_start(out=out_dram[tsl, :], in_=ob)

            def proj_chunks(i):
                return [
                    (lambda tb=tb: proj_tb(tb)) for tb in range(4 * i, 4 * i + 4)
                ]

            def project(i):
                for f in proj_chunks(i):
                    f()

            def emit_att(th, hs, proj_after=(), fillq=None):
                tbase = 1024 * th
                ilo_half, ihi_half = 2 * th, 2 * th + 2
                for h in hs:
                    rows = slice(0, 68) if h % 2 == 0 else slice(0, 128)
                    Y = {}
                    started = set()
                    for j in range(8 * th + 8):
                        i0, m = j // 4, j % 4
                        off = 128 * m
                        ilo = max(i0, ilo_half)
                        kept = [
                            i for i in range(ilo, ihi_half)
                            if 128 * j + 127 >= 512 * i - DELTA[h]
                        ]
                        if not kept:
                            continue
                        imax = kept[-1]
                        S = ps2a.tile([128, 1024], f32, tag="sc")
                        for i in kept:
                            a = 512 * i - tbase + (off if i == i0 else 0)
                            b = 512 * i - tbase + 512
                            nc.tensor.matmul(
                                S[:, a:b],
                                KP[h][rows, 128 * j:128 * (j + 1)],
                                QP[h][rows, tbase + a:tbase + b],
                                start=True,
                                stop=True,
                            )
                        amin = 512 * kept[0] - tbase + (off if kept[0] == i0 else 0)
                        amax = 512 * imax - tbase + 512
                        PT = p2pt.tile([128, 1024], bf16, tag="pt")
                        nc.scalar.activation(PT[:, amin:amax], S[:, amin:amax], EXP)
                        if i0 >= ilo_half:
                            d0 = 512 * i0 - tbase + off
                            nc.gpsimd.affine_select(
                                out=PT[:, d0:d0 + 128],
                                in_=PT[:, d0:d0 + 128],
                                compare_op=mybir.AluOpType.is_ge,
                                fill=0.0,
                                base=0,
                                pattern=[[1, 128]],
                                channel_multiplier=-1,
                            )
                        for i in sorted(kept, reverse=True):
                            if i not in Y:
                                yt = ps2b.tile(
                                    [65, 512], f32,
                                    tag=f"yb{i % 2}", name=f"Y{h}_{i}",
                                )
                                Y[i] = yt
                            a = 512 * i - tbase + (off if i == i0 else 0)
                            b = 512 * i - tbase + 512
                            ya = a - (512 * i - tbase)
                            nc.tensor.matmul(
                                Y[i][:, ya:512],
                                VP[j][:, h, :],
                                PT[:, a:b],
                                start=(i not in started),
                                stop=(j == 4 * i + 3),
                            )
                            started.add(i)
                        if j >= 3 and (j - 3) % 4 == 0:
                            i_done = (j - 3) // 4
                            if ilo_half <= i_done < ihi_half:
                                normalize(h, i_done, Y[i_done])
                                if h == hs[-1] and i_done in proj_after:
                                    if fillq is not None:
                                        fillq.extend(proj_chunks(i_done))
                                    else:
                                        project(i_done)
                        if fillq:
                            fillq.popleft()()
                if fillq:
                    while fillq:
                        fillq.popleft()()

            # --- interleaved emission ---
            emit_ts(0)
            emit_ts(1)
            emit_att(0, [0, 1])
            emit_ts(2)
            emit_att(0, [2, 3])
            emit_ts(3)
            psP.release()
            psF[0] = tc.alloc_tile_pool(name="psF", bufs=2, space="PSUM")
            # proj work drains one t-block per attention j-step so the PE
            # always has independent work while the Act engine runs exps.
            fillq = deque()
            fillq.extend(proj_chunks(0))
            fillq.extend(proj_chunks(1))
            emit_att(1, [1, 3, 2, 0], proj_after=(2, 3), fillq=fillq)
            if DEBUG_DUMP:
                for h in range(HL):
                    nc.sync.dma_start(out=qp_dump[h], in_=QP[h][:, :])
                    nc.sync.dma_start(out=kp_dump[h], in_=KP[h][:, :])
                for j in range(16):
                    nc.sync.dma_start(out=vp_dump[j], in_=VP[j][:, :, :])
                for p in range(2):
                    nc.sync.dma_start(out=pair_dump[p], in_=PAIR[p][:, :])
            psF[0].release()
            ps2b.release()
            ps2a.release()
            p3.release()
            p2pt.release()
            p2.release()

    nc.finalize()
    return nc


def _get_program():
    if "nc" not in _prog_cache:
        _prog_cache["nc"] = _build_program()
    return _prog_cache["nc"]


def _bf(a):
    return np.asarray(a, np.float32).astype(BF)


def _prep_core_inputs(core, x, w_attn, b_attn, w_proj):
    b, g = core // 4, core % 4
    # slot i holds global head g + 4*i (slopes grouped by magnitude per slot)
    heads = [g + 4 * i for i in range(HL)]
    qc = [slice((0 * H + h) * D, (0 * H + h) * D + D) for h in heads]
    kc = [slice((1 * H + h) * D, (1 * H + h) * D + D) for h in heads]
    vc = [slice((2 * H + h) * D, (2 * H + h) * D + D) for h in heads]

    wq = np.concatenate([w_attn[:, s] for s in qc], 1) * 0.125
    wk = np.concatenate([w_attn[:, s] for s in kc], 1)
    wqk = np.concatenate([wq, wk], 1).astype(np.float32)          # [C, 512]
    # [C, 512] -> [m, p, c, n] where row = c*128+p, col = m*128+n
    wqk_m = wqk.reshape(8, 128, 4, 128).transpose(2, 1, 0, 3)
    wv = np.concatenate([w_attn[:, s] for s in vc], 1).astype(np.float32)
    wv_p = wv.reshape(8, 128, 256).transpose(1, 0, 2)             # [128, 8, 256]
    bq = np.concatenate([b_attn[s] for s in qc]) * 0.125
    bk = np.concatenate([b_attn[s] for s in kc])
    bqk = np.concatenate([bq, bk]).astype(np.float32).reshape(4, 128).T.copy()
    bv = np.concatenate([b_attn[s] for s in vc]).astype(np.float32)[None, :]
    wp = np.concatenate([w_proj[s, :] for s in qc], 0).astype(np.float32)  # [256, C]
    wp_p = wp.reshape(2, 128, 1024).transpose(1, 0, 2)            # [128, 2, 1024]

    slopes = 2.0 ** (-(8.0 / H) * (np.array(heads, np.float64) + 1.0))
    pos = np.arange(T, dtype=np.float64)
    kaug = slopes[:, None] * pos[None, :]                          # [HL, T]
    khi = _bf(kaug)
    klo = _bf(kaug - khi.astype(np.float64))
    qaug = -(kaug + COFF)
    qhi = _bf(qaug)
    qlo = _bf(qaug - qhi.astype(np.float64))

    augq = np.zeros((HL, 64, T), BF)
    augq[:, 60, :] = BF(1.0)
    augq[:, 61, :] = BF(1.0)
    augq[:, 62, :] = qhi
    augq[:, 63, :] = qlo
    augk = np.zeros((HL, 64, T), BF)
    augk[:, 60, :] = khi
    augk[:, 61, :] = klo
    augk[:, 62, :] = BF(1.0)
    augk[:, 63, :] = BF(1.0)

    return {
        "xT": _bf(np.ascontiguousarray(x[b].T)),
        "wqk": _bf(np.ascontiguousarray(wqk_m)),
        "wv": _bf(np.ascontiguousarray(wv_p)),
        "wp": _bf(np.ascontiguousarray(wp_p)),
        "bqk": bqk,
        "bv": _bf(bv),
        "augq": augq,
        "augk": augk,
    }


def kernel(x, w_attn, b_attn, w_proj, b_proj, _run_kwargs=None):
    from concourse.bass_utils import run_bass_kernel_spmd

    x = np.asarray(x, np.float32)
    w_attn = np.asarray(w_attn, np.float32)
    b_attn = np.asarray(b_attn, np.float32)
    w_proj = np.asarray(w_proj, np.float32)
    b_proj = np.asarray(b_proj, np.float32)

    nc = _get_program()
    in_maps = [_prep_core_inputs(c, x, w_attn, b_attn, w_proj) for c in range(NCORES)]
    res = run_bass_kernel_spmd(
        nc, in_maps, core_ids=list(range(NCORES)), **(_run_kwargs or {})
    )
    _prog_cache["last_result"] = res

    out = np.zeros((B, T, C), np.float32)
    for c in range(NCORES):
        out[c // 4] += np.asarray(res.results[c]["out"], np.float32)
    out += b_proj[None, None, :]
    return out
